# revision 37
# baseline (speedup 1.0000x reference)
"""Trainium2 Bass kernel for nn_AttentionHead_48077863911491 (final).

Computation (per batch b of 4):
    q = h @ Wq               [S=2048, D=64]
    k = h @ Wk + bk          [S, D]
    scores = (q @ k^T) / 8   [Sq, Sk]      (1/8 folded into Wq; bq DROPPED:
                                            softmax over the query axis is
                                            exactly invariant to bq.(k+bk))
    w = softmax(scores, axis=0)            # over the QUERY axis
    out = w @ h              [Sq, E=1024]  # h (not v) is the value tensor

Sharding: 8 cores = 4 batches x 2 key-halves; half=1 cores get h rolled by
-1024 so keys are always rows 0:1024 (identical SPMD program); the host
rolls the partial back and sums the two key-half partials (f32).

Schedule (TimelineSim 84383 ns/core vs 96804 ns baseline; HW-validated):
  - hT streams in chunk-major (4 query chunks of 512) with chunks 0/1 as
    separate piece-tiles: projections consume pieces as they land, so the
    whole left edge is DMA-limited (~340 GB/s model), not PE-limited.
  - PACKED projection: stationary [wq*SCALE | wk] per e-tile gives Q^T on
    PSUM partitions 0:64 and K^T on 64:128 in one 8-matmul pass per chunk
    (32 proj matmuls total).  K^T evicts with a partition-shifted
    tensor_scalar_add(+bk) to rows 0:64 (walrus/HW-verified), in per-
    key-tile pieces so the first scores matmul unblocks early; scores then
    contract over just 64 partitions - no zero-padding.
  - softmax: exp at quarter granularity on ACT; chunk-0/1 exps carry
    accum_out per-key sums, chunk-2/3 exps use DVE tensor_reduce instead
    (keeps the ACT tail stream short).  ACT order: [c0 x kt][c1 kt0-1]
    then per key-tile [c2, c3] pairs with the remaining c1 exps as filler,
    so key-tiles finalize incrementally from ~17us while ACT stays busy.
    Normalization is folded into the value rows (hs = hk * 1/sum, DVE).
  - phase C (out^T = sum_k hs^T expw): a 6-bank riding wave of PSUM
    accumulators consumes each key-tile as it finalizes (per iteration the
    next key-tile scores/exp/fin are emitted BEFORE the ride block); the
    remaining 26 accumulators then run at full PE speed, evictions
    alternating DVE/ACT, f16 output DMA; the last accumulator is split
    into two half-width accumulations so its eviction+DMA chain overlaps
    the final matmuls.
"""

import numpy as np

import concourse.bass as bass
import concourse.mybir as mybir
import concourse.tile as tile
from concourse import bacc
from concourse.bass_utils import run_bass_kernel_spmd

B, S, E, D = 4, 2048, 1024, 64
KH = S // 2          # keys per core
P = 128
ET = E // P          # 8 e-tiles
KT = KH // P         # 8 key-tiles per core
QC = S // 512        # 4 query chunks of 512
SCALE = 1.0 / np.sqrt(D)

_cached = {}


def build_bass(reps=1, c_mult=1):
    f16, f32 = mybir.dt.float16, mybir.dt.float32
    Exp = mybir.ActivationFunctionType.Exp
    AX = mybir.AxisListType.X
    ADD = mybir.AluOpType.add
    nc = bacc.Bacc("TRN2", target_bir_lowering=False, debug=False, num_devices=8)

    hT = nc.dram_tensor("hT", [E, S], f16, kind="ExternalInput").ap()
    hk = nc.dram_tensor("hk", [KH, E], f16, kind="ExternalInput").ap()
    # packed stationary: [p, et, 0:64]=wq*SCALE row et*128+p, [p, et, 64:128]=wk
    wqk = nc.dram_tensor("wqk", [P, ET * P], f16, kind="ExternalInput").ap()
    bk = nc.dram_tensor("bk", [D, 1], f32, kind="ExternalInput").ap()
    outT = nc.dram_tensor("outT", [E, S], f16, kind="ExternalOutput").ap()

    hT4 = hT.rearrange("(t p) (c q) -> c p t q", p=P, q=512)
    hk3 = hk.rearrange("(t p) e -> t p e", p=P)      # [8, 128, 1024]
    outT3 = outT.rearrange("(t p) q -> t p q", p=P)  # [8, 128, 2048]

    with tile.TileContext(nc) as tc:
        with (
            tc.tile_pool(name="p_w", bufs=1) as p_w,
            tc.tile_pool(name="p_in", bufs=1) as p_in,
            tc.tile_pool(name="p_soft", bufs=1) as p_soft,
            tc.tile_pool(name="p_out", bufs=10) as p_out,
        ):
            for _rep in range(reps):
                # ---- input DMAs (SP seq; chunk-major, piece-tiled early) ----
                wqk_sb = p_w.tile([P, ET, P], f16, tag="wqk")
                nc.sync.dma_start(
                    wqk_sb[:].rearrange("p t d -> p (t d)"), wqk[:])
                bk_sb = p_w.tile([D, 1], f32, tag="bk")
                nc.sync.dma_start(bk_sb[:], bk[:])
                # chunk 0: four 2-et piece tiles; chunk 1: two 4-et pieces;
                # chunks 2/3: whole tiles.  (et -> (tile, sub))
                hpiece = {}
                for i in range(4):
                    t = p_in.tile([P, 2, 512], f16, tag=f"h0p{i}",
                                  name=f"h0p{i}")
                    nc.sync.dma_start(t[:], hT4[0][:, 2 * i:2 * i + 2, :])
                    for s in range(2):
                        hpiece[(0, 2 * i + s)] = t[:, s, :]
                for i in range(2):
                    t = p_in.tile([P, 4, 512], f16, tag=f"h1p{i}",
                                  name=f"h1p{i}")
                    nc.sync.dma_start(t[:], hT4[1][:, 4 * i:4 * i + 4, :])
                    for s in range(4):
                        hpiece[(1, 4 * i + s)] = t[:, s, :]
                for c in (2, 3):
                    t = p_in.tile([P, ET, 512], f16, tag=f"hT{c}",
                                  name=f"hT{c}")
                    nc.sync.dma_start(t[:], hT4[c])
                    for s in range(ET):
                        hpiece[(c, s)] = t[:, s, :]
                hk_sb = []
                for kt in range(KT):
                    t = p_in.tile([P, E], f16, tag=f"hk{kt}", name=f"hk{kt}")
                    nc.sync.dma_start(t[:], hk3[kt])
                    hk_sb.append(t)

                QT16 = p_w.tile([D, S], f16, tag="qt")
                KT16 = p_w.tile([D, KH], f16, tag="kt")
                expw = [p_soft.tile([P, S], f16, tag=f"ew{kt}", name=f"ew{kt}")
                        for kt in range(KT)]
                hs = [p_soft.tile([P, E], f16, tag=f"hs{kt}", name=f"hs{kt}")
                      for kt in range(KT)]
                ssumq = p_w.tile([P, 4 * KT], f32, tag="ssumq")
                rsum = p_w.tile([P, KT], f32, tag="rsum")

                sc_pend = {}

                with tc.tile_pool(name="ps_sc", bufs=2, space="PSUM") as ps_sc:

                    def emit_sc(kt, c):
                        t = ps_sc.tile([P, 512], f32, tag="sc",
                                       name=f"sc{kt}_{c}")
                        nc.tensor.matmul(
                            t[:], KT16[:, kt * P:(kt + 1) * P],
                            QT16[:, c * 512:(c + 1) * 512],
                            start=True, stop=True)
                        sc_pend[(kt, c)] = t

                    def emit_exp(kt, c):
                        accum = c < 2
                        if accum:
                            nc.scalar.activation(
                                expw[kt][:, c * 512:(c + 1) * 512],
                                sc_pend.pop((kt, c))[:], Exp,
                                accum_out=ssumq[:, kt * 4 + c:kt * 4 + c + 1])
                        else:
                            nc.scalar.activation(
                                expw[kt][:, c * 512:(c + 1) * 512],
                                sc_pend.pop((kt, c))[:], Exp)
                            nc.vector.tensor_reduce(
                                ssumq[:, kt * 4 + c:kt * 4 + c + 1],
                                expw[kt][:, c * 512:(c + 1) * 512], AX, ADD)

                    def emit_fin(kt):
                        nc.vector.tensor_reduce(
                            rsum[:, kt:kt + 1], ssumq[:, kt * 4:kt * 4 + 4],
                            AX, ADD)
                        nc.vector.reciprocal_approx_fast(
                            rsum[:, kt:kt + 1], rsum[:, kt:kt + 1])
                        nc.vector.tensor_scalar_mul(
                            hs[kt][:], hk_sb[kt][:], rsum[:, kt:kt + 1])

                    with tc.tile_pool(name="ps_p", bufs=2, space="PSUM") as ps_p:

                        def emit_proj(c):
                            # packed: Q^T on rows 0:64, K^T on rows 64:128
                            PP = ps_p.tile([P, 512], f32, tag="pp",
                                           name=f"pp{c}")
                            for et in range(ET):
                                nc.tensor.matmul(
                                    PP[:], wqk_sb[:, et, :], hpiece[(c, et)],
                                    start=(et == 0), stop=(et == ET - 1))
                            nc.vector.tensor_copy(
                                QT16[:, c * 512:(c + 1) * 512], PP[0:D, :])
                            if c < 2:
                                # partition-shifted eviction rows 64:128 ->
                                # KT16 rows 0:64, +bk; per-key-tile pieces so
                                # the first scores matmul unblocks early
                                for s in range(4):
                                    nc.vector.tensor_scalar_add(
                                        KT16[:, c * 512 + s * P:
                                             c * 512 + (s + 1) * P],
                                        PP[D:P, s * P:(s + 1) * P], bk_sb[:])

                        # ---- phase A/B: PE emission follows the ACT
                        # consumption order (sc ring = 3) ----
                        emit_proj(0)
                        emit_sc(0, 0)
                        emit_exp(0, 0)
                        emit_sc(1, 0)
                        emit_exp(1, 0)
                        emit_proj(1)
                        for kt in (2, 3, 4, 5, 6, 7):
                            emit_sc(kt, 0)
                            emit_exp(kt, 0)
                        emit_sc(0, 1)
                        emit_exp(0, 1)
                        emit_sc(1, 1)
                        emit_exp(1, 1)
                        emit_proj(2)
                        emit_sc(0, 2)
                        emit_exp(0, 2)
                        emit_proj(3)
                        emit_sc(0, 3)
                        emit_exp(0, 3)
                        emit_fin(0)
                        emit_sc(1, 2)
                        emit_exp(1, 2)
                        emit_sc(1, 3)
                        emit_exp(1, 3)
                        emit_fin(1)
                        emit_sc(2, 1)
                        emit_exp(2, 1)

                    # ---- phase C ----
                    with tc.tile_pool(name="ps_c", bufs=6, space="PSUM") as ps_c:
                        accs = {}

                        def emit_cmm(a, kt):
                            et, qc = a
                            nc.tensor.matmul(
                                accs[a][:],
                                hs[kt][:, et * P:(et + 1) * P],
                                expw[kt][:, qc * 512:(qc + 1) * 512],
                                start=(kt == 0), stop=(kt == KT - 1))

                        ev_engines = [nc.vector.tensor_copy, nc.scalar.copy]

                        def emit_evict(a, idx):
                            et, qc = a
                            ot = p_out.tile([P, 512], f16, tag="ot",
                                            name=f"ot{et}_{qc}")
                            ev_engines[idx % 2](ot[:], accs[a][:])
                            nc.sync.dma_start(
                                outT3[et][:, qc * 512:(qc + 1) * 512], ot[:])

                        # riding wave: 5 accumulators; the remaining kt-tail
                        # scores/exps (kt>=2 c2/c3, c1 fillers kt>=3) are
                        # interleaved between the kt ride-blocks in the same
                        # order ACT consumes them.
                        W0 = [(et, 0) for et in range(6)]
                        for a in W0:
                            accs[a] = ps_c.tile([P, 512], f32, tag="acc",
                                                name=f"acc{a[0]}_{a[1]}")
                        for kt in range(KT):
                            j = kt + 2
                            if j < KT:
                                emit_sc(j, 2)
                                emit_exp(j, 2)
                                emit_sc(j, 3)
                                emit_exp(j, 3)
                                emit_fin(j)
                            j = kt + 3
                            if j < KT:
                                emit_sc(j, 1)
                                emit_exp(j, 1)
                            for a in W0:
                                emit_cmm(a, kt)
                        for i, a in enumerate(W0):
                            emit_evict(a, i)

                        rest = [(et, qc) for qc in range(1, QC)
                                for et in range(ET)] + [(6, 0), (7, 0)]
                        for _extra in range(c_mult - 1):
                            rest = rest + rest  # timing probe
                        for i, a in enumerate(rest):
                            last = i == len(rest) - 1
                            if not last:
                                accs[a] = ps_c.tile([P, 512], f32, tag="acc",
                                                    name=f"acc{a[0]}_{a[1]}_{i}")
                                for kt in range(KT):
                                    emit_cmm(a, kt)
                                emit_evict(a, i)
                                continue
                            # last accumulator: two independent half-width
                            # accumulations so eviction/DMA overlaps the tail
                            et, qc = a
                            halves = [ps_c.tile([P, 512], f32, tag="acc",
                                                name=f"acch{hh}")
                                      for hh in range(2)]
                            ot = p_out.tile([P, 512], f16, tag="ot",
                                            name="ot_last")
                            bounds = [(0, 256), (256, 512)]
                            for hh, ha in enumerate(halves):
                                lo, hi = bounds[hh]
                                w = hi - lo
                                for kt in range(KT):
                                    nc.tensor.matmul(
                                        ha[:, 0:w],
                                        hs[kt][:, et * P:(et + 1) * P],
                                        expw[kt][:, qc * 512 + lo:
                                                  qc * 512 + hi],
                                        start=(kt == 0), stop=(kt == KT - 1))
                                eng = nc.scalar.copy if hh == 0 else \
                                    nc.vector.tensor_copy
                                eng(ot[:, lo:hi], ha[:, 0:w])
                                nc.sync.dma_start(
                                    outT3[et][:, qc * 512 + lo:
                                              qc * 512 + hi],
                                    ot[:, lo:hi])

    nc.compile()
    return nc


def _prep_in_maps(h, Wq, bq, Wk, bk):
    wq = (np.asarray(Wq, np.float32) * SCALE).astype(np.float16)
    wk = np.asarray(Wk, np.float32).astype(np.float16)
    # [E, D] each -> packed [P, ET, 128]: (p, et, 0:64)=wq, (p, et, 64:128)=wk
    packed = np.concatenate(
        [wq.reshape(ET, P, D), wk.reshape(ET, P, D)], axis=2)  # [ET, P, 128]
    wqk = np.ascontiguousarray(
        packed.transpose(1, 0, 2).reshape(P, ET * P))
    bk_col = np.ascontiguousarray(np.asarray(bk, np.float32).reshape(D, 1))
    in_maps = []
    for c in range(8):
        b, half = divmod(c, 2)
        hb = np.asarray(h[b], np.float32)
        rolled = np.roll(hb, -KH * half, axis=0) if half else hb
        h16 = rolled.astype(np.float16)
        in_maps.append({
            "hT": np.ascontiguousarray(h16.T),
            "hk": np.ascontiguousarray(h16[0:KH]),
            "wqk": wqk, "bk": bk_col,
        })
    return in_maps


def _assemble(results):
    out = np.empty((B, S, E), np.float32)
    for b in range(B):
        p0 = results[2 * b]["outT"].T.astype(np.float32)
        p1 = results[2 * b + 1]["outT"].T.astype(np.float32)
        out[b] = p0 + np.roll(p1, KH, axis=0)
    return out


def kernel(h, Wq, bq, Wk, bk, Wv=None, bv=None, **_unused):
    if "nc" not in _cached:
        _cached["nc"] = build_bass()
    nc = _cached["nc"]
    in_maps = _prep_in_maps(h, Wq, bq, Wk, bk)
    res = run_bass_kernel_spmd(nc, in_maps, list(range(8)))
    return _assemble(res.results)


# revision 40
# speedup vs baseline: 1.0032x; 1.0032x over previous
"""Trainium2 Bass kernel for nn_AttentionHead_48077863911491 (final).

Computation (per batch b of 4):
    q = h @ Wq               [S=2048, D=64]
    k = h @ Wk + bk          [S, D]
    scores = (q @ k^T) / 8   [Sq, Sk]      (1/8 folded into Wq; bq DROPPED:
                                            softmax over the query axis is
                                            exactly invariant to bq.(k+bk))
    w = softmax(scores, axis=0)            # over the QUERY axis
    out = w @ h              [Sq, E=1024]  # h (not v) is the value tensor

Sharding: 8 cores = 4 batches x 2 key-halves; half=1 cores get h rolled by
-1024 so keys are always rows 0:1024 (identical SPMD program); the host
rolls the partial back and sums the two key-half partials (f32).

Schedule (TimelineSim 84383 ns/core vs 96804 ns baseline; HW-validated):
  - hT streams in chunk-major (4 query chunks of 512) with chunks 0/1 as
    separate piece-tiles: projections consume pieces as they land, so the
    whole left edge is DMA-limited (~340 GB/s model), not PE-limited.
  - PACKED projection: stationary [wq*SCALE | wk] per e-tile gives Q^T on
    PSUM partitions 0:64 and K^T on 64:128 in one 8-matmul pass per chunk
    (32 proj matmuls total).  K^T evicts with a partition-shifted
    tensor_scalar_add(+bk) to rows 0:64 (walrus/HW-verified), in per-
    key-tile pieces so the first scores matmul unblocks early; scores then
    contract over just 64 partitions - no zero-padding.
  - softmax: exp at quarter granularity on ACT; chunk-0/1 exps carry
    accum_out per-key sums, chunk-2/3 exps use DVE tensor_reduce instead
    (keeps the ACT tail stream short).  ACT order: [c0 x kt][c1 kt0-1]
    then per key-tile [c2, c3] pairs with the remaining c1 exps as filler,
    so key-tiles finalize incrementally from ~17us while ACT stays busy.
    Normalization is folded into the value rows (hs = hk * 1/sum, DVE).
  - phase C (out^T = sum_k hs^T expw): a 6-bank riding wave of PSUM
    accumulators consumes each key-tile as it finalizes (per iteration the
    next key-tile scores/exp/fin are emitted BEFORE the ride block); the
    remaining 26 accumulators then run at full PE speed, evictions
    alternating DVE/ACT, f16 output DMA; the last accumulator is split
    into two half-width accumulations so its eviction+DMA chain overlaps
    the final matmuls.
"""

import numpy as np

import concourse.bass as bass
import concourse.mybir as mybir
import concourse.tile as tile
from concourse import bacc
from concourse.bass_utils import run_bass_kernel_spmd

B, S, E, D = 4, 2048, 1024, 64
KH = S // 2          # keys per core
P = 128
ET = E // P          # 8 e-tiles
KT = KH // P         # 8 key-tiles per core
QC = S // 512        # 4 query chunks of 512
SCALE = 1.0 / np.sqrt(D)

_cached = {}


def build_bass(reps=1, c_mult=1):
    f16, f32 = mybir.dt.float16, mybir.dt.float32
    Exp = mybir.ActivationFunctionType.Exp
    AX = mybir.AxisListType.X
    ADD = mybir.AluOpType.add
    nc = bacc.Bacc("TRN2", target_bir_lowering=False, debug=False, num_devices=8)

    hT = nc.dram_tensor("hT", [E, S], f16, kind="ExternalInput").ap()
    hk = nc.dram_tensor("hk", [KH, E], f16, kind="ExternalInput").ap()
    # packed stationary: [p, et, 0:64]=wq*SCALE row et*128+p, [p, et, 64:128]=wk
    wqk = nc.dram_tensor("wqk", [P, ET * P], f16, kind="ExternalInput").ap()
    bk = nc.dram_tensor("bk", [D, 1], f32, kind="ExternalInput").ap()
    outT = nc.dram_tensor("outT", [E, S], f16, kind="ExternalOutput").ap()

    hT4 = hT.rearrange("(t p) (c q) -> c p t q", p=P, q=512)
    hk3 = hk.rearrange("(t p) e -> t p e", p=P)      # [8, 128, 1024]
    outT3 = outT.rearrange("(t p) q -> t p q", p=P)  # [8, 128, 2048]

    with tile.TileContext(nc) as tc:
        with (
            tc.tile_pool(name="p_w", bufs=1) as p_w,
            tc.tile_pool(name="p_in", bufs=1) as p_in,
            tc.tile_pool(name="p_soft", bufs=1) as p_soft,
            tc.tile_pool(name="p_out", bufs=10) as p_out,
        ):
            for _rep in range(reps):
                # ---- input DMAs (SP seq; chunk-major, piece-tiled early) ----
                wqk_sb = p_w.tile([P, ET, P], f16, tag="wqk")
                nc.sync.dma_start(
                    wqk_sb[:].rearrange("p t d -> p (t d)"), wqk[:])
                bk_sb = p_w.tile([D, 1], f32, tag="bk")
                nc.sync.dma_start(bk_sb[:], bk[:])
                # chunk 0: four 2-et piece tiles; chunk 1: two 4-et pieces;
                # chunks 2/3: whole tiles.  (et -> (tile, sub))
                hpiece = {}
                for i in range(4):
                    t = p_in.tile([P, 2, 512], f16, tag=f"h0p{i}",
                                  name=f"h0p{i}")
                    nc.sync.dma_start(t[:], hT4[0][:, 2 * i:2 * i + 2, :])
                    for s in range(2):
                        hpiece[(0, 2 * i + s)] = t[:, s, :]
                for i in range(2):
                    t = p_in.tile([P, 4, 512], f16, tag=f"h1p{i}",
                                  name=f"h1p{i}")
                    nc.sync.dma_start(t[:], hT4[1][:, 4 * i:4 * i + 4, :])
                    for s in range(4):
                        hpiece[(1, 4 * i + s)] = t[:, s, :]
                for c in (2, 3):
                    t = p_in.tile([P, ET, 512], f16, tag=f"hT{c}",
                                  name=f"hT{c}")
                    nc.sync.dma_start(t[:], hT4[c])
                    for s in range(ET):
                        hpiece[(c, s)] = t[:, s, :]
                hk_sb = []
                for kt in range(KT):
                    t = p_in.tile([P, E], f16, tag=f"hk{kt}", name=f"hk{kt}")
                    nc.sync.dma_start(t[:], hk3[kt])
                    hk_sb.append(t)

                QT16 = p_w.tile([D, S], f16, tag="qt")
                KT16 = p_w.tile([D, KH], f16, tag="kt")
                expw = [p_soft.tile([P, S], f16, tag=f"ew{kt}", name=f"ew{kt}")
                        for kt in range(KT)]
                # hs split into two half-tiles per key-tile: the first
                # riding matmuls unblock after the 512-col multiply
                hs = [[p_soft.tile([P, E // 2], f16, tag=f"hs{kt}_{hh}",
                                   name=f"hs{kt}_{hh}") for hh in range(2)]
                      for kt in range(KT)]
                ssumq = p_w.tile([P, 4 * KT], f32, tag="ssumq")
                rsum = p_w.tile([P, KT], f32, tag="rsum")

                sc_pend = {}

                with tc.tile_pool(name="ps_sc", bufs=2, space="PSUM") as ps_sc:

                    def emit_sc(kt, c):
                        t = ps_sc.tile([P, 512], f32, tag="sc",
                                       name=f"sc{kt}_{c}")
                        nc.tensor.matmul(
                            t[:], KT16[:, kt * P:(kt + 1) * P],
                            QT16[:, c * 512:(c + 1) * 512],
                            start=True, stop=True)
                        sc_pend[(kt, c)] = t

                    def emit_exp(kt, c):
                        accum = c < 2
                        if accum:
                            nc.scalar.activation(
                                expw[kt][:, c * 512:(c + 1) * 512],
                                sc_pend.pop((kt, c))[:], Exp,
                                accum_out=ssumq[:, kt * 4 + c:kt * 4 + c + 1])
                        else:
                            nc.scalar.activation(
                                expw[kt][:, c * 512:(c + 1) * 512],
                                sc_pend.pop((kt, c))[:], Exp)
                            nc.vector.tensor_reduce(
                                ssumq[:, kt * 4 + c:kt * 4 + c + 1],
                                expw[kt][:, c * 512:(c + 1) * 512], AX, ADD)

                    def emit_fin(kt):
                        nc.vector.tensor_reduce(
                            rsum[:, kt:kt + 1], ssumq[:, kt * 4:kt * 4 + 4],
                            AX, ADD)
                        nc.vector.reciprocal_approx_fast(
                            rsum[:, kt:kt + 1], rsum[:, kt:kt + 1])
                        for hh in range(2):
                            nc.vector.tensor_scalar_mul(
                                hs[kt][hh][:],
                                hk_sb[kt][:, hh * (E // 2):(hh + 1) * (E // 2)],
                                rsum[:, kt:kt + 1])

                    with tc.tile_pool(name="ps_p", bufs=2, space="PSUM") as ps_p:

                        def emit_proj(c):
                            # packed: Q^T on rows 0:64, K^T on rows 64:128
                            PP = ps_p.tile([P, 512], f32, tag="pp",
                                           name=f"pp{c}")
                            for et in range(ET):
                                nc.tensor.matmul(
                                    PP[:], wqk_sb[:, et, :], hpiece[(c, et)],
                                    start=(et == 0), stop=(et == ET - 1))
                            nc.vector.tensor_copy(
                                QT16[:, c * 512:(c + 1) * 512], PP[0:D, :])
                            if c < 2:
                                # partition-shifted eviction rows 64:128 ->
                                # KT16 rows 0:64, +bk; per-key-tile pieces so
                                # the first scores matmul unblocks early
                                for s in range(4):
                                    nc.vector.tensor_scalar_add(
                                        KT16[:, c * 512 + s * P:
                                             c * 512 + (s + 1) * P],
                                        PP[D:P, s * P:(s + 1) * P], bk_sb[:])

                        # ---- phase A/B: PE emission follows the ACT
                        # consumption order (sc ring = 3) ----
                        emit_proj(0)
                        emit_sc(0, 0)
                        emit_exp(0, 0)
                        emit_sc(1, 0)
                        emit_exp(1, 0)
                        emit_proj(1)
                        for kt in (2, 3, 4, 5, 6, 7):
                            emit_sc(kt, 0)
                            emit_exp(kt, 0)
                        emit_sc(0, 1)
                        emit_exp(0, 1)
                        emit_sc(1, 1)
                        emit_exp(1, 1)
                        emit_proj(2)
                        emit_sc(0, 2)
                        emit_exp(0, 2)
                        emit_proj(3)
                        emit_sc(0, 3)
                        emit_exp(0, 3)
                        emit_fin(0)
                        emit_sc(1, 2)
                        emit_exp(1, 2)
                        emit_sc(1, 3)
                        emit_exp(1, 3)
                        emit_fin(1)
                        emit_sc(2, 1)
                        emit_exp(2, 1)

                    # ---- phase C ----
                    with tc.tile_pool(name="ps_c", bufs=6, space="PSUM") as ps_c:
                        accs = {}

                        def emit_cmm(a, kt):
                            et, qc = a
                            nc.tensor.matmul(
                                accs[a][:],
                                hs[kt][et // 4][:, (et % 4) * P:
                                                (et % 4 + 1) * P],
                                expw[kt][:, qc * 512:(qc + 1) * 512],
                                start=(kt == 0), stop=(kt == KT - 1))

                        ev_engines = [nc.vector.tensor_copy, nc.scalar.copy]

                        def emit_evict(a, idx):
                            et, qc = a
                            ot = p_out.tile([P, 512], f16, tag="ot",
                                            name=f"ot{et}_{qc}")
                            ev_engines[idx % 2](ot[:], accs[a][:])
                            nc.sync.dma_start(
                                outT3[et][:, qc * 512:(qc + 1) * 512], ot[:])

                        # riding wave: 5 accumulators; the remaining kt-tail
                        # scores/exps (kt>=2 c2/c3, c1 fillers kt>=3) are
                        # interleaved between the kt ride-blocks in the same
                        # order ACT consumes them.
                        W0 = [(et, 0) for et in range(6)]
                        for a in W0:
                            accs[a] = ps_c.tile([P, 512], f32, tag="acc",
                                                name=f"acc{a[0]}_{a[1]}")
                        for kt in range(KT):
                            j = kt + 2
                            if j < KT:
                                emit_sc(j, 2)
                                emit_exp(j, 2)
                                emit_sc(j, 3)
                                emit_exp(j, 3)
                                emit_fin(j)
                            j = kt + 3
                            if j < KT:
                                emit_sc(j, 1)
                                emit_exp(j, 1)
                            for a in W0:
                                emit_cmm(a, kt)
                        for i, a in enumerate(W0):
                            emit_evict(a, i)

                        rest = [(et, qc) for qc in range(1, QC)
                                for et in range(ET)] + [(6, 0), (7, 0)]
                        for _extra in range(c_mult - 1):
                            rest = rest + rest  # timing probe
                        for i, a in enumerate(rest):
                            last = i == len(rest) - 1
                            if not last:
                                accs[a] = ps_c.tile([P, 512], f32, tag="acc",
                                                    name=f"acc{a[0]}_{a[1]}_{i}")
                                for kt in range(KT):
                                    emit_cmm(a, kt)
                                emit_evict(a, i)
                                continue
                            # last accumulator: two independent half-width
                            # accumulations so eviction/DMA overlaps the tail
                            et, qc = a
                            halves = [ps_c.tile([P, 512], f32, tag="acc",
                                                name=f"acch{hh}")
                                      for hh in range(2)]
                            ot = p_out.tile([P, 512], f16, tag="ot",
                                            name="ot_last")
                            bounds = [(0, 256), (256, 512)]
                            for hh, ha in enumerate(halves):
                                lo, hi = bounds[hh]
                                w = hi - lo
                                for kt in range(KT):
                                    nc.tensor.matmul(
                                        ha[:, 0:w],
                                        hs[kt][et // 4][:, (et % 4) * P:
                                                        (et % 4 + 1) * P],
                                        expw[kt][:, qc * 512 + lo:
                                                  qc * 512 + hi],
                                        start=(kt == 0), stop=(kt == KT - 1))
                                eng = nc.scalar.copy if hh == 0 else \
                                    nc.vector.tensor_copy
                                eng(ot[:, lo:hi], ha[:, 0:w])
                                nc.sync.dma_start(
                                    outT3[et][:, qc * 512 + lo:
                                              qc * 512 + hi],
                                    ot[:, lo:hi])

    nc.compile()
    return nc


def _prep_in_maps(h, Wq, bq, Wk, bk):
    wq = (np.asarray(Wq, np.float32) * SCALE).astype(np.float16)
    wk = np.asarray(Wk, np.float32).astype(np.float16)
    # [E, D] each -> packed [P, ET, 128]: (p, et, 0:64)=wq, (p, et, 64:128)=wk
    packed = np.concatenate(
        [wq.reshape(ET, P, D), wk.reshape(ET, P, D)], axis=2)  # [ET, P, 128]
    wqk = np.ascontiguousarray(
        packed.transpose(1, 0, 2).reshape(P, ET * P))
    bk_col = np.ascontiguousarray(np.asarray(bk, np.float32).reshape(D, 1))
    in_maps = []
    for c in range(8):
        b, half = divmod(c, 2)
        hb = np.asarray(h[b], np.float32)
        rolled = np.roll(hb, -KH * half, axis=0) if half else hb
        h16 = rolled.astype(np.float16)
        in_maps.append({
            "hT": np.ascontiguousarray(h16.T),
            "hk": np.ascontiguousarray(h16[0:KH]),
            "wqk": wqk, "bk": bk_col,
        })
    return in_maps


def _assemble(results):
    out = np.empty((B, S, E), np.float32)
    for b in range(B):
        p0 = results[2 * b]["outT"].T.astype(np.float32)
        p1 = results[2 * b + 1]["outT"].T.astype(np.float32)
        out[b] = p0 + np.roll(p1, KH, axis=0)
    return out


def kernel(h, Wq, bq, Wk, bk, Wv=None, bv=None, **_unused):
    if "nc" not in _cached:
        _cached["nc"] = build_bass()
    nc = _cached["nc"]
    in_maps = _prep_in_maps(h, Wq, bq, Wk, bk)
    res = run_bass_kernel_spmd(nc, in_maps, list(range(8)))
    return _assemble(res.results)
